# revision 11
# baseline (speedup 1.0000x reference)
"""Trainium2 Bass kernel for nn_Diagonal: out = x * abs(diag(W)).

Pure data-parallel: x [65536, 1024] is sharded along batch across 8
NeuronCores (8192 rows each); W [1024, 1024] is replicated. Each core:
  1. DMAs the 1024 diagonal elements of W (stride D+1) into SBUF,
  2. abs() them, broadcasts across all 128 partitions via a K=1
     ones-matmul on the PE (PSUM), copies PSUM->SBUF,
  3. streams x through SBUF in 8 tiles of [128, 8192] f32 (4 MB each),
     multiplying by the broadcast diagonal on the vector engine.
Memory-bound: 32 MB in + 32 MB out per core.
"""

from contextlib import ExitStack

import numpy as np

import concourse.bacc as bacc
import concourse.bass as bass
import concourse.mybir as mybir
import concourse.tile as tile
from concourse.bass_utils import run_bass_kernel_spmd

N_CORES = 8
B, D = 65536, 1024
B_SHARD = B // N_CORES  # 8192
P = 128
TILE_FD = 2048  # [128, 2048] f32 = 1 MB per tile
F = TILE_FD // D  # rows of x per partition per tile
N_TILES = B_SHARD // (P * F)
X_BUFS = 16  # deep rotation keeps both DMA streams continuously fed
MM_N = 512  # one PSUM bank per matmul

_cached_nc = None


def _build():
    nc = bacc.Bacc(
        "TRN2", target_bir_lowering=False, debug=False, num_devices=N_CORES
    )
    x_t = nc.dram_tensor("x", [B_SHARD, D], mybir.dt.float32, kind="ExternalInput")
    w_t = nc.dram_tensor("W", [D, D], mybir.dt.float32, kind="ExternalInput")
    o_t = nc.dram_tensor("out", [B_SHARD, D], mybir.dt.float32, kind="ExternalOutput")
    x, W, out = x_t.ap(), w_t.ap(), o_t.ap()

    NB = D // P  # 8 diagonal [128,128] blocks of W
    x3 = x.rearrange("(n p f) d -> n p (f d)", p=P, f=F)
    o3 = out.rearrange("(n p f) d -> n p (f d)", p=P, f=F)

    with tile.TileContext(nc) as tc, ExitStack() as ctx:
        const_pool = ctx.enter_context(tc.tile_pool(name="const", bufs=1))
        xpool = ctx.enter_context(tc.tile_pool(name="x", bufs=X_BUFS))
        pspool = ctx.enter_context(tc.tile_pool(name="ps", bufs=1, space="PSUM"))

        # Load the 8 diagonal [128,128] blocks of W in one HWDGE DMA
        # (512B-contiguous descriptors -- avoids the 13us GPSIMD descriptor
        # generation a 1025-strided element gather would cost).
        blocks = const_pool.tile([P, D], mybir.dt.float32)
        for a in range(NB):
            nc.scalar.dma_start(
                out=blocks[:, a * P : (a + 1) * P],
                in_=W[a * P : (a + 1) * P, a * P : (a + 1) * P],
            )
        b3 = blocks[:, :].rearrange("p (a q) -> p a q", q=P)

        # identity mask: keep only Waa[p, p]
        eye = const_pool.tile([P, P], mybir.dt.float32)
        nc.gpsimd.memset(eye[:, :], 0.0)
        nc.gpsimd.affine_select(
            out=eye[:, :],
            in_=eye[:, :],
            compare_op=mybir.AluOpType.not_equal,
            fill=1.0,
            base=0,
            pattern=[[-1, P]],
            channel_multiplier=1,
        )
        masked = const_pool.tile([P, D], mybir.dt.float32)
        eye_b = eye[:, :].unsqueeze(1).broadcast_to((P, NB, P))
        nc.vector.tensor_tensor(
            masked[:, :].rearrange("p (a q) -> p a q", q=P), b3, eye_b,
            mybir.AluOpType.mult,
        )

        # column-sum via ones-matmul: broadcasts diag across all partitions
        ones = const_pool.tile([P, P], mybir.dt.float32)
        nc.vector.memset(ones[:, :], 1.0)
        ps = pspool.tile([P, D], mybir.dt.float32)
        for j in range(D // MM_N):
            nc.tensor.matmul(
                ps[:, j * MM_N : (j + 1) * MM_N],
                lhsT=ones[:, :],
                rhs=masked[:, j * MM_N : (j + 1) * MM_N],
                start=True,
                stop=True,
            )
        # abs fused into the PSUM->SBUF copy (one-hot column sums, so abs
        # commutes with the sum)
        drep = const_pool.tile([P, D], mybir.dt.float32)
        nc.scalar.activation(
            drep[:, :], ps[:, :], mybir.ActivationFunctionType.Abs
        )
        dbb = drep[:, :].unsqueeze(1).broadcast_to((P, F, D))

        for i in range(N_TILES):
            xt = xpool.tile([P, TILE_FD], mybir.dt.float32)
            nc.sync.dma_start(out=xt[:, :], in_=x3[i])
            x3d = xt[:, :].rearrange("p (f d) -> p f d", d=D)
            nc.vector.tensor_tensor(x3d, x3d, dbb, mybir.AluOpType.mult)
            nc.scalar.dma_start(out=o3[i], in_=xt[:, :])
    nc.compile()
    return nc


def _get_nc():
    global _cached_nc
    if _cached_nc is None:
        _cached_nc = _build()
    return _cached_nc


def run(x, W, **run_kwargs):
    """Shard, execute on 8 cores, gather. Returns (output, BassKernelResults)."""
    x = np.ascontiguousarray(np.asarray(x, dtype=np.float32))
    W = np.ascontiguousarray(np.asarray(W, dtype=np.float32))
    assert x.shape == (B, D) and W.shape == (D, D)
    nc = _get_nc()
    in_maps = [
        {"x": x[i * B_SHARD : (i + 1) * B_SHARD], "W": W} for i in range(N_CORES)
    ]
    res = run_bass_kernel_spmd(nc, in_maps, list(range(N_CORES)), **run_kwargs)
    full = np.concatenate([r["out"] for r in res.results], axis=0)
    return full, res


def kernel(x, W):
    return run(x, W)[0]
